# revision 7
# baseline (speedup 1.0000x reference)
"""Trainium2 Bass kernel: 2D valid cross-correlation (3x3) + bias on 8192x8192 fp32.

Strategy:
  - Row-shard X across 8 NeuronCores with a 2-row halo handled by host-side
    overlapped slicing (each core gets a 1026x8192 slab; core 7's slab is
    shifted up by 2 rows so all cores run an identical SPMD program).
  - bf16 end-to-end on the wire: the host casts X to bf16 (halves the HBM
    read), the device computes bf16 matmuls accumulating in fp32 PSUM, and
    the output is written back as bf16 (halves the HBM write); the host
    upcasts to fp32. Quantization error ~3e-3 l2-relative, well under the
    2e-2 gate. Per-core HBM traffic ~33.6MB -> ~94us at the 358 GB/s
    per-core HBM limit, vs ~188us for fp32.
  - On each core: conv2d(3x3) = 3 PSUM-accumulating matmuls per output tile
    with banded "shift" matrices built from the 3x3 weight. The banded matrix
    B_dj[p, o] = w[p - o, dj] turns the partition-axis (row) shifts into a
    matmul contraction; the column shifts dj are free-axis offsets of the rhs.
    The bands are padded to 128 stationary columns so the compiler's Fast
    Weight Load path (requires NumWeights==128) halves LDWEIGHTS cost; the
    two garbage output partitions are simply not evacuated.
  - PSUM evacuation (fused bias add + fp32->bf16 downcast) is split between
    DVE (tensor_scalar_add) and ACT (activation Identity with per-partition
    bias AP), 2+2 col-tiles per group, so evacuation never gates the PE's
    PSUM bank reuse.
  - The 16-row tail slab uses 4-way PE column tiling (tile_position) so its
    matmuls run concurrently in disjoint 32-col groups of the array: ~3us
    instead of a full ~10us stream pass.
  - Loads ride the sync HWDGE ring in ~1MB chunks (first chunk of slab 0 is
    small so compute starts early); stores ride the scalar HWDGE ring in
    ~1MB half-slabs.
"""

import os
import sys

import numpy as np
import ml_dtypes

for _p in (
    "/opt/trn_rl_repo",
    "/root/.axon_site/_ro/trn_rl_repo",
    "/root/.axon_site/_ro/pypackages",
    "/opt/pypackages",
):
    if os.path.isdir(_p) and _p not in sys.path:
        sys.path.append(_p)

import concourse.bacc as bacc
import concourse.mybir as mybir
import concourse.tile as tile
from concourse.bass_utils import run_bass_kernel_spmd

BF16 = ml_dtypes.bfloat16

N_CORES = 8
H = W = 8192
KH = KW = 3
OH = H - KH + 1  # 8190
OW = W - KW + 1  # 8190
ROWS_PER_CORE = 1024  # output rows produced per core (core 7: first 2 dropped)
SLAB_IN_ROWS = 1026  # input rows per core slab
SLAB_IN = 128  # input rows per row-slab tile
SLAB_OUT = 126  # output rows per row-slab tile
BAND_COLS = 128  # stationary columns (padded from 126 -> 128 for FWL)
N_FULL_SLABS = 8  # 8 * 126 = 1008
TAIL_IN = SLAB_IN_ROWS - N_FULL_SLABS * SLAB_OUT  # 18
TAIL_OUT = ROWS_PER_CORE - N_FULL_SLABS * SLAB_OUT  # 16
COL_TILE = 512
N_COLT = (OW + COL_TILE - 1) // COL_TILE  # 16 (15*512 + 510)
GROUP = 4  # col-tiles per dj-outer matmul group (= PSUM banks per group)
N_GROUPS = N_COLT // GROUP  # 4

# Column chunking for DMA: ~1MB chunks (bf16) keep transfers at line rate
# while the pipeline gets fine-grained dependencies. Slab 0's first chunk is
# the minimum needed by the first matmul group so compute starts early.
IN_CHUNKS = [(0, 4098), (4098, 8192)]
IN_CHUNKS_FIRST = [(0, 514), (514, 2050), (2050, 4098), (4098, 8192)]
# Stores go per matmul group (512KB) so the store stream tracks compute
# closely and the post-compute drain is one group, not a whole slab.
OUT_CHUNKS = [(0, 2048), (2048, 4096), (4096, 6144), (6144, 8190)]
WARMUP_MMS = 12  # ~5us of dummy matmuls to flip HAM to 8/8 before real work

_NC = None
LAST_RESULTS = None


def _build():
    nc = bacc.Bacc(
        "TRN2", target_bir_lowering=False, debug=False, num_devices=N_CORES
    )
    bf16 = mybir.dt.bfloat16
    f32 = mybir.dt.float32
    act_id = mybir.ActivationFunctionType.Identity

    xs = nc.dram_tensor("xs", [SLAB_IN_ROWS, W], bf16, kind="ExternalInput")
    bands = nc.dram_tensor(
        "bands", [SLAB_IN, KW, BAND_COLS], bf16, kind="ExternalInput"
    )
    biasb = nc.dram_tensor("biasb", [SLAB_IN, 1], f32, kind="ExternalInput")
    out = nc.dram_tensor(
        "out", [ROWS_PER_CORE, OW], bf16, kind="ExternalOutput"
    )

    with tile.TileContext(nc) as tc:
        with (
            tc.tile_pool(name="const", bufs=1) as cpool,
            tc.tile_pool(name="inp", bufs=3) as ipool,
            tc.tile_pool(name="outp", bufs=2) as opool,
            tc.tile_pool(name="psum", bufs=2 * GROUP, space="PSUM") as pspool,
        ):
            # PE warmup: the HAM clock gate keeps the PE at 1.2 GHz until it
            # has been busy ~3.4us. The first ~10us of the kernel are DMA
            # setup anyway, so run dummy matmuls on a zeroed scratch tile to
            # flip the gate to 2.4 GHz before the first real matmul arrives.
            wt = cpool.tile([SLAB_IN, 128 + COL_TILE], bf16, tag="warm")
            nc.gpsimd.memset(wt[:], 0)
            wps = pspool.tile([BAND_COLS, COL_TILE], f32, tag="ps", name="warm")
            for _ in range(WARMUP_MMS):
                nc.tensor.matmul(
                    wps[:], wt[:, :128], wt[:, 128:], start=True, stop=True
                )

            # Consts ride the sync (HWDGE) ring ahead of the first slab chunks:
            # tiny transfers that gate the first matmul, so they go first.
            bt = cpool.tile([SLAB_IN, KW, BAND_COLS], bf16, tag="bt")
            nc.sync.dma_start(bt[:], bands.ap())
            bias_t = cpool.tile([SLAB_IN, 1], f32, tag="bias")
            nc.sync.dma_start(bias_t[:], biasb.ap())

            for s in range(N_FULL_SLABS + 1):
                tail = s == N_FULL_SLABS
                in_rows = TAIL_IN if tail else SLAB_IN
                out_rows = TAIL_OUT if tail else SLAB_OUT
                r0 = s * SLAB_OUT

                it = ipool.tile([SLAB_IN, W], bf16, tag="it", name=f"it{s}")
                for a, b in IN_CHUNKS_FIRST if s == 0 else IN_CHUNKS:
                    nc.sync.dma_start(
                        it[:in_rows, a:b], xs.ap()[r0 : r0 + in_rows, a:b]
                    )

                ot = opool.tile([SLAB_OUT, OW], bf16, tag="ot", name=f"ot{s}")

                if not tail:
                    for g in range(N_GROUPS):
                        pss = [
                            pspool.tile(
                                [BAND_COLS, COL_TILE],
                                f32,
                                tag="ps",
                                name=f"ps{s}_{g}_{t}",
                            )
                            for t in range(GROUP)
                        ]
                        for dj in range(KW):
                            for t in range(GROUP):
                                j = GROUP * g + t
                                c0 = j * COL_TILE
                                n = min(COL_TILE, OW - c0)
                                nc.tensor.matmul(
                                    pss[t][:, :n],
                                    bt[:, dj, :],
                                    it[:, c0 + dj : c0 + dj + n],
                                    start=(dj == 0),
                                    stop=(dj == KW - 1),
                                )
                        # Evacuate PSUM -> SBUF with bias add + bf16 downcast,
                        # split 2+2 across DVE and ACT so neither gates the PE.
                        for t in range(GROUP):
                            j = GROUP * g + t
                            c0 = j * COL_TILE
                            n = min(COL_TILE, OW - c0)
                            if t < 2:
                                nc.vector.tensor_scalar_add(
                                    ot[:out_rows, c0 : c0 + n],
                                    pss[t][:out_rows, :n],
                                    bias_t[:out_rows, :],
                                )
                            else:
                                nc.scalar.activation(
                                    ot[:out_rows, c0 : c0 + n],
                                    pss[t][:out_rows, :n],
                                    act_id,
                                    bias=bias_t[:out_rows, :],
                                )
                        a, b = OUT_CHUNKS[g]
                        nc.scalar.dma_start(
                            out.ap()[r0 : r0 + out_rows, a:b],
                            ot[:out_rows, a:b],
                        )
                else:
                    # Tail: 16 output rows. Pack 4 col-tiles into disjoint
                    # 32-wide column groups of the PE array so their streams
                    # run concurrently (~4x), instead of paying a full-width
                    # stream pass for 16 rows.
                    for g in range(N_GROUPS):
                        ps = pspool.tile(
                            [BAND_COLS, COL_TILE], f32, tag="ps", name=f"pst{g}"
                        )
                        for dj in range(KW):
                            for t in range(GROUP):
                                j = GROUP * g + t
                                c0 = j * COL_TILE
                                n = min(COL_TILE, OW - c0)
                                nc.tensor.matmul(
                                    ps[32 * t : 32 * t + out_rows, :n],
                                    bt[:in_rows, dj, :out_rows],
                                    it[:in_rows, c0 + dj : c0 + dj + n],
                                    start=(dj == 0),
                                    stop=(dj == KW - 1),
                                    tile_position=(0, 32 * t),
                                )
                        for t in range(GROUP):
                            j = GROUP * g + t
                            c0 = j * COL_TILE
                            n = min(COL_TILE, OW - c0)
                            if t < 2:
                                nc.vector.tensor_scalar_add(
                                    ot[:out_rows, c0 : c0 + n],
                                    ps[32 * t : 32 * t + out_rows, :n],
                                    bias_t[:out_rows, :],
                                )
                            else:
                                nc.scalar.activation(
                                    ot[:out_rows, c0 : c0 + n],
                                    ps[32 * t : 32 * t + out_rows, :n],
                                    act_id,
                                    bias=bias_t[:out_rows, :],
                                )
                        a, b = OUT_CHUNKS[g]
                        nc.scalar.dma_start(
                            out.ap()[r0 : r0 + out_rows, a:b],
                            ot[:out_rows, a:b],
                        )

    nc.compile()
    return nc


def kernel(X, weight, bias):
    global _NC, LAST_RESULTS
    X = np.asarray(X, dtype=np.float32)
    weight = np.asarray(weight, dtype=np.float32)
    bias = np.asarray(bias, dtype=np.float32).reshape(-1)

    if _NC is None:
        _NC = _build()
    nc = _NC

    Xb = np.ascontiguousarray(X.astype(BF16))

    # Banded shift matrices: bands[p, dj, o] = w[p - o, dj] for 0 <= p-o < 3.
    # Columns 126/127 are FWL padding; their outputs are garbage and ignored.
    bands = np.zeros((SLAB_IN, KW, BAND_COLS), dtype=np.float32)
    o = np.arange(SLAB_OUT)
    for di in range(KH):
        for dj in range(KW):
            bands[o + di, dj, o] = weight[di, dj]
    bands = bands.astype(BF16)
    biasb = np.full((SLAB_IN, 1), bias[0], dtype=np.float32)

    starts = [min(i * ROWS_PER_CORE, H - SLAB_IN_ROWS) for i in range(N_CORES)]
    in_maps = [
        {
            "xs": Xb[s0 : s0 + SLAB_IN_ROWS],
            "bands": bands,
            "biasb": biasb,
        }
        for s0 in starts
    ]

    # The shared device occasionally returns corrupted results after an NRT
    # wedge (observed once across dozens of runs: rel err jumped ~12 orders of
    # magnitude on an unchanged binary). A handful of sampled rows checked
    # against a host conv (~2M flops) catches that reliably — the legit bf16
    # error is ~1e-1 absolute at |Y|~20 while corruption shows up as O(10+)
    # relative — so retry the device run when the spot check fails.
    for attempt in range(3):
        res = run_bass_kernel_spmd(nc, in_maps, core_ids=list(range(N_CORES)))
        LAST_RESULTS = res

        full = np.empty((OH, OW), dtype=np.float32)
        for i in range(N_CORES - 1):
            full[i * ROWS_PER_CORE : (i + 1) * ROWS_PER_CORE] = res.results[i][
                "out"
            ].astype(np.float32)
        # Core 7's slab starts at row 7166, so its first 2 output rows
        # duplicate core 6's last 2; keep rows 2.. (= conv rows 7168..8189).
        full[(N_CORES - 1) * ROWS_PER_CORE :] = res.results[N_CORES - 1]["out"][
            ROWS_PER_CORE - (OH - (N_CORES - 1) * ROWS_PER_CORE) :
        ].astype(np.float32)
        if _spot_check(full, X, weight, bias[0]):
            return full
        print(
            f"kernel: device output failed spot check (attempt {attempt + 1}); "
            "retrying",
            file=sys.stderr,
        )
    return full


def _spot_check(full, X, w, bias):
    rows = set()
    for i in range(N_CORES):
        base = i * ROWS_PER_CORE
        rows.update((base, base + 513, base + SLAB_OUT * 4, base + 1010, base + 1023))
    rows.add(OH - 1)
    for r in sorted(rows):
        if r >= OH:
            continue
        ref = np.zeros(OW, dtype=np.float32)
        for di in range(KH):
            for dj in range(KW):
                ref += w[di, dj] * X[r + di, dj : dj + OW]
        ref += bias
        tol = max(0.05 * float(np.abs(ref).max()), 0.05)
        if float(np.abs(full[r] - ref).max()) > tol:
            return False
    return True


# revision 10
# speedup vs baseline: 1.0131x; 1.0131x over previous
"""Trainium2 Bass kernel: 2D valid cross-correlation (3x3) + bias on 8192x8192 fp32.

Strategy:
  - Row-shard X across 8 NeuronCores with a 2-row halo handled by host-side
    overlapped slicing (each core gets a 1026x8192 slab; core 7's slab is
    shifted up by 2 rows so all cores run an identical SPMD program).
  - bf16 end-to-end on the wire: the host casts X to bf16 (halves the HBM
    read), the device computes bf16 matmuls accumulating in fp32 PSUM, and
    the output is written back as bf16 (halves the HBM write); the host
    upcasts to fp32. Quantization error ~3e-3 l2-relative, well under the
    2e-2 gate. Per-core HBM traffic ~33.6MB -> ~94us at the 358 GB/s
    per-core HBM limit, vs ~188us for fp32.
  - On each core: conv2d(3x3) = 3 PSUM-accumulating matmuls per output tile
    with banded "shift" matrices built from the 3x3 weight. The banded matrix
    B_dj[p, o] = w[p - o, dj] turns the partition-axis (row) shifts into a
    matmul contraction; the column shifts dj are free-axis offsets of the rhs.
    The bands are padded to 128 stationary columns so the compiler's Fast
    Weight Load path (requires NumWeights==128) halves LDWEIGHTS cost; the
    two garbage output partitions are simply not evacuated.
  - PSUM evacuation (fused bias add + fp32->bf16 downcast) is split between
    DVE (tensor_scalar_add) and ACT (activation Identity with per-partition
    bias AP), 2+2 col-tiles per group, so evacuation never gates the PE's
    PSUM bank reuse.
  - The 16-row tail slab uses 4-way PE column tiling (tile_position) so its
    matmuls run concurrently in disjoint 32-col groups of the array: ~3us
    instead of a full ~10us stream pass.
  - Loads ride the sync HWDGE ring in ~1MB chunks (first chunk of slab 0 is
    small so compute starts early); stores ride the scalar HWDGE ring in
    ~1MB half-slabs.
"""

import os
import sys

import numpy as np
import ml_dtypes

for _p in (
    "/opt/trn_rl_repo",
    "/root/.axon_site/_ro/trn_rl_repo",
    "/root/.axon_site/_ro/pypackages",
    "/opt/pypackages",
):
    if os.path.isdir(_p) and _p not in sys.path:
        sys.path.append(_p)

import concourse.bacc as bacc
import concourse.mybir as mybir
import concourse.tile as tile
from concourse.bass_utils import run_bass_kernel_spmd

BF16 = ml_dtypes.bfloat16

N_CORES = 8
H = W = 8192
KH = KW = 3
OH = H - KH + 1  # 8190
OW = W - KW + 1  # 8190
ROWS_PER_CORE = 1024  # output rows produced per core (core 7: first 2 dropped)
SLAB_IN_ROWS = 1026  # input rows per core slab
SLAB_IN = 128  # input rows per row-slab tile
SLAB_OUT = 126  # output rows per row-slab tile
BAND_COLS = 128  # stationary columns (padded from 126 -> 128 for FWL)
N_FULL_SLABS = 8  # 8 * 126 = 1008
TAIL_IN = SLAB_IN_ROWS - N_FULL_SLABS * SLAB_OUT  # 18
TAIL_OUT = ROWS_PER_CORE - N_FULL_SLABS * SLAB_OUT  # 16
COL_TILE = 512
N_COLT = (OW + COL_TILE - 1) // COL_TILE  # 16 (15*512 + 510)
GROUP = 4  # col-tiles per dj-outer matmul group (= PSUM banks per group)
N_GROUPS = N_COLT // GROUP  # 4

# Column chunking for DMA: ~1MB chunks (bf16) keep transfers at line rate
# while the pipeline gets fine-grained dependencies. Slab 0's first chunk is
# the minimum needed by the first matmul group so compute starts early.
IN_CHUNKS = [(0, 4098), (4098, 8192)]
IN_CHUNKS_FIRST = [(0, 514), (514, 4098), (4098, 8192)]
# Stores go per matmul group (512KB) so the store stream tracks compute
# closely and the post-compute drain is one group, not a whole slab. They
# are issued on the otherwise-idle GpSimd engine (SWDGE ring) so store
# descriptor generation never competes with ACT's PSUM evacuation work.
OUT_CHUNKS = [(0, 2048), (2048, 4096), (4096, 6144), (6144, 8190)]
WARMUP_MMS = 10  # ~4us of dummy matmuls to flip HAM to 8/8 before real work

_NC = None
LAST_RESULTS = None


def _build():
    nc = bacc.Bacc(
        "TRN2", target_bir_lowering=False, debug=False, num_devices=N_CORES
    )
    bf16 = mybir.dt.bfloat16
    f32 = mybir.dt.float32
    act_id = mybir.ActivationFunctionType.Identity

    xs = nc.dram_tensor("xs", [SLAB_IN_ROWS, W], bf16, kind="ExternalInput")
    bands = nc.dram_tensor(
        "bands", [SLAB_IN, KW, BAND_COLS], bf16, kind="ExternalInput"
    )
    biasb = nc.dram_tensor("biasb", [SLAB_IN, 1], f32, kind="ExternalInput")
    out = nc.dram_tensor(
        "out", [ROWS_PER_CORE, OW], bf16, kind="ExternalOutput"
    )

    with tile.TileContext(nc) as tc:
        with (
            tc.tile_pool(name="const", bufs=1) as cpool,
            tc.tile_pool(name="inp", bufs=3) as ipool,
            tc.tile_pool(name="outp", bufs=2) as opool,
            tc.tile_pool(name="psum", bufs=2 * GROUP, space="PSUM") as pspool,
        ):
            # PE warmup: the HAM clock gate keeps the PE at 1.2 GHz until it
            # has been busy ~3.4us. The first ~10us of the kernel are DMA
            # setup anyway, so run dummy matmuls on a zeroed scratch tile to
            # flip the gate to 2.4 GHz before the first real matmul arrives.
            wt = cpool.tile([SLAB_IN, 128 + COL_TILE], bf16, tag="warm")
            nc.vector.memset(wt[:], 0)
            wps = pspool.tile([BAND_COLS, COL_TILE], f32, tag="ps", name="warm")
            for _ in range(WARMUP_MMS):
                nc.tensor.matmul(
                    wps[:], wt[:, :128], wt[:, 128:], start=True, stop=True
                )

            # Consts ride the sync (HWDGE) ring ahead of the first slab chunks:
            # tiny transfers that gate the first matmul, so they go first.
            bt = cpool.tile([SLAB_IN, KW, BAND_COLS], bf16, tag="bt")
            nc.sync.dma_start(bt[:], bands.ap())
            bias_t = cpool.tile([SLAB_IN, 1], f32, tag="bias")
            nc.sync.dma_start(bias_t[:], biasb.ap())

            for s in range(N_FULL_SLABS + 1):
                tail = s == N_FULL_SLABS
                in_rows = TAIL_IN if tail else SLAB_IN
                out_rows = TAIL_OUT if tail else SLAB_OUT
                r0 = s * SLAB_OUT

                it = ipool.tile([SLAB_IN, W], bf16, tag="it", name=f"it{s}")
                for a, b in IN_CHUNKS_FIRST if s == 0 else IN_CHUNKS:
                    nc.sync.dma_start(
                        it[:in_rows, a:b], xs.ap()[r0 : r0 + in_rows, a:b]
                    )

                ot = opool.tile([SLAB_OUT, OW], bf16, tag="ot", name=f"ot{s}")

                if not tail:
                    for g in range(N_GROUPS):
                        pss = [
                            pspool.tile(
                                [BAND_COLS, COL_TILE],
                                f32,
                                tag="ps",
                                name=f"ps{s}_{g}_{t}",
                            )
                            for t in range(GROUP)
                        ]
                        for dj in range(KW):
                            for t in range(GROUP):
                                j = GROUP * g + t
                                c0 = j * COL_TILE
                                n = min(COL_TILE, OW - c0)
                                nc.tensor.matmul(
                                    pss[t][:, :n],
                                    bt[:, dj, :],
                                    it[:, c0 + dj : c0 + dj + n],
                                    start=(dj == 0),
                                    stop=(dj == KW - 1),
                                )
                        # Evacuate PSUM -> SBUF with bias add + bf16 downcast,
                        # split 2+2 across DVE and ACT so neither gates the PE.
                        for t in range(GROUP):
                            j = GROUP * g + t
                            c0 = j * COL_TILE
                            n = min(COL_TILE, OW - c0)
                            if t < 2:
                                nc.vector.tensor_scalar_add(
                                    ot[:out_rows, c0 : c0 + n],
                                    pss[t][:out_rows, :n],
                                    bias_t[:out_rows, :],
                                )
                            else:
                                nc.scalar.activation(
                                    ot[:out_rows, c0 : c0 + n],
                                    pss[t][:out_rows, :n],
                                    act_id,
                                    bias=bias_t[:out_rows, :],
                                )
                        a, b = OUT_CHUNKS[g]
                        nc.gpsimd.dma_start(
                            out.ap()[r0 : r0 + out_rows, a:b],
                            ot[:out_rows, a:b],
                        )
                else:
                    # Tail: 16 output rows. Pack 4 col-tiles into disjoint
                    # 32-wide column groups of the PE array so their streams
                    # run concurrently (~4x), instead of paying a full-width
                    # stream pass for 16 rows.
                    for g in range(N_GROUPS):
                        ps = pspool.tile(
                            [BAND_COLS, COL_TILE], f32, tag="ps", name=f"pst{g}"
                        )
                        for dj in range(KW):
                            for t in range(GROUP):
                                j = GROUP * g + t
                                c0 = j * COL_TILE
                                n = min(COL_TILE, OW - c0)
                                nc.tensor.matmul(
                                    ps[32 * t : 32 * t + out_rows, :n],
                                    bt[:in_rows, dj, :out_rows],
                                    it[:in_rows, c0 + dj : c0 + dj + n],
                                    start=(dj == 0),
                                    stop=(dj == KW - 1),
                                    tile_position=(0, 32 * t),
                                )
                        for t in range(GROUP):
                            j = GROUP * g + t
                            c0 = j * COL_TILE
                            n = min(COL_TILE, OW - c0)
                            if t < 2:
                                nc.vector.tensor_scalar_add(
                                    ot[:out_rows, c0 : c0 + n],
                                    ps[32 * t : 32 * t + out_rows, :n],
                                    bias_t[:out_rows, :],
                                )
                            else:
                                nc.scalar.activation(
                                    ot[:out_rows, c0 : c0 + n],
                                    ps[32 * t : 32 * t + out_rows, :n],
                                    act_id,
                                    bias=bias_t[:out_rows, :],
                                )
                        a, b = OUT_CHUNKS[g]
                        nc.gpsimd.dma_start(
                            out.ap()[r0 : r0 + out_rows, a:b],
                            ot[:out_rows, a:b],
                        )

    nc.compile()
    return nc


def kernel(X, weight, bias):
    global _NC, LAST_RESULTS
    X = np.asarray(X, dtype=np.float32)
    weight = np.asarray(weight, dtype=np.float32)
    bias = np.asarray(bias, dtype=np.float32).reshape(-1)

    if _NC is None:
        _NC = _build()
    nc = _NC

    Xb = np.ascontiguousarray(X.astype(BF16))

    # Banded shift matrices: bands[p, dj, o] = w[p - o, dj] for 0 <= p-o < 3.
    # Columns 126/127 are FWL padding; their outputs are garbage and ignored.
    bands = np.zeros((SLAB_IN, KW, BAND_COLS), dtype=np.float32)
    o = np.arange(SLAB_OUT)
    for di in range(KH):
        for dj in range(KW):
            bands[o + di, dj, o] = weight[di, dj]
    bands = bands.astype(BF16)
    biasb = np.full((SLAB_IN, 1), bias[0], dtype=np.float32)

    starts = [min(i * ROWS_PER_CORE, H - SLAB_IN_ROWS) for i in range(N_CORES)]
    in_maps = [
        {
            "xs": Xb[s0 : s0 + SLAB_IN_ROWS],
            "bands": bands,
            "biasb": biasb,
        }
        for s0 in starts
    ]

    # The shared device occasionally returns corrupted results after an NRT
    # wedge (observed once across dozens of runs: rel err jumped ~12 orders of
    # magnitude on an unchanged binary). A handful of sampled rows checked
    # against a host conv (~2M flops) catches that reliably — the legit bf16
    # error is ~1e-1 absolute at |Y|~20 while corruption shows up as O(10+)
    # relative — so retry the device run when the spot check fails.
    for attempt in range(3):
        res = run_bass_kernel_spmd(nc, in_maps, core_ids=list(range(N_CORES)))
        LAST_RESULTS = res

        full = np.empty((OH, OW), dtype=np.float32)
        for i in range(N_CORES - 1):
            full[i * ROWS_PER_CORE : (i + 1) * ROWS_PER_CORE] = res.results[i][
                "out"
            ].astype(np.float32)
        # Core 7's slab starts at row 7166, so its first 2 output rows
        # duplicate core 6's last 2; keep rows 2.. (= conv rows 7168..8189).
        full[(N_CORES - 1) * ROWS_PER_CORE :] = res.results[N_CORES - 1]["out"][
            ROWS_PER_CORE - (OH - (N_CORES - 1) * ROWS_PER_CORE) :
        ].astype(np.float32)
        if _spot_check(full, X, weight, bias[0]):
            return full
        print(
            f"kernel: device output failed spot check (attempt {attempt + 1}); "
            "retrying",
            file=sys.stderr,
        )
    return full


def _spot_check(full, X, w, bias):
    rows = set()
    for i in range(N_CORES):
        base = i * ROWS_PER_CORE
        rows.update((base, base + 513, base + SLAB_OUT * 4, base + 1010, base + 1023))
    rows.add(OH - 1)
    for r in sorted(rows):
        if r >= OH:
            continue
        ref = np.zeros(OW, dtype=np.float32)
        for di in range(KH):
            for dj in range(KW):
                ref += w[di, dj] * X[r + di, dj : dj + OW]
        ref += bias
        tol = max(0.05 * float(np.abs(ref).max()), 0.05)
        if float(np.abs(full[r] - ref).max()) > tol:
            return False
    return True
